# revision 6
# baseline (speedup 1.0000x reference)
"""MultiHeadAttention Trainium2 kernel (B=4, S=2048, D=1024, H=16, Dh=64).

Sharding: 8 cores = 4 batches x 2 head-groups (8 heads each).  Each core
computes QKV projections for its (batch, head-group), full attention for its
8 heads, and a partial output projection (row-parallel over Wo).  The host
sums the two per-batch partials and adds the output bias.

Per-core dataflow (all matmuls bf16 inputs, fp32 PSUM accumulation):
  - inputs arrive pre-transposed: qT/kT/vT [D, S], weight slices transposed.
  - QT/KT [512, S] computed directly in transposed (head-dim major) layout.
  - V [S, 512] computed in natural layout, stored with a ones-column per head
    (V_ext [s, h, 65]) so the attention-value matmul also produces the
    softmax denominator.
  - scoresT[k, q] = K_h @ Q_h^T   (contraction over head dim, K=64)
  - expT = exp(scoresT / 8) on ScalarE (no max subtraction: |scores/8| < ~7)
  - ctxT_ext[65, q] = V_ext^T @ expT accumulated over k tiles; row 64 is the
    denominator.  reciprocal -> PE broadcast over 64 partitions -> multiply.
  - out_partial[s, 1024] accumulated per head (K=64) from ctxT and WoT.
"""

import sys

for _p in ("/opt/trn_rl_repo", "/root/.axon_site/_ro/trn_rl_repo"):
    if _p not in sys.path:
        sys.path.append(_p)

import numpy as np
import ml_dtypes

import concourse.bass as bass
import concourse.tile as tile
from concourse import bacc, mybir
from concourse.bass_utils import run_bass_kernel_spmd

BF16 = ml_dtypes.bfloat16
F32 = mybir.dt.float32
BF = mybir.dt.bfloat16

D_MODEL = 1024
NUM_HEADS = 16
HEAD_DIM = 64
B, S = 4, 2048
HPC = 8          # heads per core
DHG = HPC * HEAD_DIM  # 512, head dims per core

# knobs read by test.py
TRACE = False
TRACE_CORES = None
LAST_RESULT = None

_PROGRAM_CACHE = {}


def _build_program(kt: int) -> bass.Bass:
    """Build the per-core SPMD program. kt = contraction tiles over d_model
    (8 normally, 9 when inputs are augmented with a bias row)."""
    nc = bacc.Bacc("TRN2", debug=False, target_bir_lowering=False)

    kd = kt * 128
    qT = nc.dram_tensor("qT", [kd, S], BF, kind="ExternalInput").ap()
    kT = nc.dram_tensor("kT", [kd, S], BF, kind="ExternalInput").ap()
    vT = nc.dram_tensor("vT", [kd, S], BF, kind="ExternalInput").ap()
    wqT = nc.dram_tensor("wqT", [kd, DHG], BF, kind="ExternalInput").ap()
    wkT = nc.dram_tensor("wkT", [kd, DHG], BF, kind="ExternalInput").ap()
    wvT = nc.dram_tensor("wvT", [kd, DHG], BF, kind="ExternalInput").ap()
    woTh = nc.dram_tensor("woTh", [HEAD_DIM, HPC, D_MODEL], BF,
                          kind="ExternalInput").ap()
    out = nc.dram_tensor("out", [S, D_MODEL], F32, kind="ExternalOutput").ap()

    with tile.TileContext(nc) as tc:
        _body(tc, qT, kT, vT, wqT, wkT, wvT, woTh, out, kt)
    nc.compile()
    return nc


def _body(tc, qT, kT, vT, wqT, wkT, wvT, woTh, out, kt):
    nc = tc.nc
    EXP = mybir.ActivationFunctionType.Exp

    with (
        tc.tile_pool(name="consts", bufs=1) as consts,
        tc.tile_pool(name="bigs", bufs=1) as bigs,
    ):
        # ---- resident tensors ------------------------------------------------
        sb_wq = consts.tile([128, kt, DHG], BF)
        sb_wk = consts.tile([128, kt, DHG], BF)
        sb_wv = consts.tile([128, kt, DHG], BF)
        sb_wo = consts.tile([HEAD_DIM, HPC, D_MODEL], BF)
        sb_ones = consts.tile([128, 64], F32)
        nc.sync.dma_start(sb_wq, wqT.rearrange("(t p) m -> p t m", p=128))
        nc.sync.dma_start(sb_wk, wkT.rearrange("(t p) m -> p t m", p=128))
        nc.sync.dma_start(sb_wv, wvT.rearrange("(t p) m -> p t m", p=128))
        nc.sync.dma_start(sb_wo, woTh)
        nc.vector.memset(sb_ones, 1.0)

        sb_QT = bigs.tile([128, 4, S], BF)    # [dh%128, dh//128, s]
        sb_KT = bigs.tile([128, 4, S], BF)
        sb_V = bigs.tile([128, 16, HPC, 65], BF)  # [s%128, s//128, h, dh+den]
        sb_ctxT = bigs.tile([64, HPC, S], BF)     # [dh%64, h, s]
        nc.vector.memset(sb_V[:, :, :, 64:65], 1.0)

        # ---- stage B: projections -------------------------------------------
        # Q and K: out[dh-tile 128, s] ; contraction over d_model.
        for name, w_t, src, dst, eng in (
            ("q", sb_wq, qT, sb_QT, nc.scalar),
            ("k", sb_wk, kT, sb_KT, nc.vector),
        ):
            with (
                tc.tile_pool(name=f"ld_{name}", bufs=2 * kt) as loads,
                tc.tile_pool(name=f"ps_{name}", bufs=8, space="PSUM") as psums,
            ):
                for nbp in range(2):            # s blocks of 1024
                    ps = [psums.tile([128, 512], F32, tag="pp", name=f"pp{name}{nbp}_{i}")
                          for i in range(8)]
                    for ki in range(kt):
                        t_in = loads.tile([128, 1024], BF, tag="ld")
                        nc.gpsimd.dma_start(
                            t_in,
                            src[ki * 128:(ki + 1) * 128,
                                nbp * 1024:(nbp + 1) * 1024])
                        for nb in range(2):
                            for g in range(4):
                                nc.tensor.matmul(
                                    ps[nb * 4 + g],
                                    lhsT=w_t[:, ki, g * 128:(g + 1) * 128],
                                    rhs=t_in[:, nb * 512:(nb + 1) * 512],
                                    start=(ki == 0), stop=(ki == kt - 1))
                    for nb in range(2):
                        for g in range(4):
                            c0 = nbp * 1024 + nb * 512
                            if eng is nc.scalar:
                                eng.copy(out=dst[:, g, c0:c0 + 512],
                                         in_=ps[nb * 4 + g])
                            else:
                                eng.tensor_copy(out=dst[:, g, c0:c0 + 512],
                                                in_=ps[nb * 4 + g])

        # V: out[s-tile 128, dh 512] ; lhsT = input tile (stationary).
        with (
            tc.tile_pool(name="ld_v", bufs=2 * kt) as loads,
            tc.tile_pool(name="ps_v", bufs=8, space="PSUM") as psums,
        ):
            for nbp in range(2):
                ps = [psums.tile([128, 512], F32, tag="pv", name=f"pv{nbp}_{i}") for i in range(8)]
                for ki in range(kt):
                    t_in = loads.tile([128, 1024], BF, tag="lv")
                    nc.gpsimd.dma_start(
                        t_in,
                        vT[ki * 128:(ki + 1) * 128,
                           nbp * 1024:(nbp + 1) * 1024])
                    for sti in range(8):
                        nc.tensor.matmul(
                            ps[sti],
                            lhsT=t_in[:, sti * 128:(sti + 1) * 128],
                            rhs=sb_wv[:, ki, :],
                            start=(ki == 0), stop=(ki == kt - 1))
                for sti in range(8):
                    st = nbp * 8 + sti
                    nc.vector.tensor_copy(
                        out=sb_V[:, st, :, 0:64],
                        in_=ps[sti].rearrange("p (h d) -> p h d", h=HPC))

        # ---- stage C: attention ---------------------------------------------
        with (
            tc.tile_pool(name="exps", bufs=6) as exps,
            tc.tile_pool(name="smalls", bufs=2) as smalls,
            tc.tile_pool(name="ps_sc", bufs=2, space="PSUM") as ps_sc_pool,
            tc.tile_pool(name="ps_cx", bufs=2, space="PSUM") as ps_cx_pool,
            tc.tile_pool(name="ps_bc", bufs=1, space="PSUM") as ps_bc_pool,
        ):
            for h in range(HPC):
                pi0 = (h % 2) * 64
                g = h // 2
                for qb in range(4):          # query blocks of 512
                    q0 = qb * 512
                    ps_ctx = ps_cx_pool.tile([128, 512], F32, tag="ctx")
                    for kp in range(8):      # pairs of key tiles
                        ps_sc = ps_sc_pool.tile([128, 2, 512], F32, tag="sc")
                        for j in range(2):
                            ktile = kp * 2 + j
                            nc.tensor.matmul(
                                ps_sc[:, j, :],
                                lhsT=sb_KT[pi0:pi0 + 64, g,
                                           ktile * 128:(ktile + 1) * 128],
                                rhs=sb_QT[pi0:pi0 + 64, g, q0:q0 + 512],
                                start=True, stop=True)
                        t_e = exps.tile([128, 2, 512], BF, tag="exp")
                        nc.scalar.activation(t_e, ps_sc, EXP, scale=0.125)
                        for j in range(2):
                            ktile = kp * 2 + j
                            nc.tensor.matmul(
                                ps_ctx[0:65, :],
                                lhsT=sb_V[:, ktile, h, :],
                                rhs=t_e[:, j, :],
                                start=(ktile == 0), stop=(ktile == 15))
                    # normalize: row 64 of ps_ctx is the denominator
                    t_rd = smalls.tile([128, 512], F32, tag="rd")
                    nc.vector.reciprocal(t_rd[64:65, :], ps_ctx[64:65, :])
                    ps_b = ps_bc_pool.tile([64, 512], F32, tag="bc")
                    nc.tensor.matmul(
                        ps_b,
                        lhsT=sb_ones[64:65, 0:64],
                        rhs=t_rd[64:65, :],
                        start=True, stop=True)
                    t_rdb = smalls.tile([64, 512], F32, tag="rdb")
                    nc.vector.tensor_copy(out=t_rdb, in_=ps_b)
                    nc.vector.tensor_mul(
                        out=sb_ctxT[0:64, h, q0:q0 + 512],
                        in0=ps_ctx[0:64, :],
                        in1=t_rdb)

        # ---- stage D: output projection (partial; host adds the halves) -----
        with (
            tc.tile_pool(name="outst", bufs=16) as outs_pool,
            tc.tile_pool(name="ps_o", bufs=4, space="PSUM") as ps_o_pool,
        ):
            for st in range(16):
                t_out = outs_pool.tile([128, D_MODEL], F32, tag="o")
                for nb2 in range(2):
                    ps_o = ps_o_pool.tile([128, 512], F32, tag="po")
                    for h in range(HPC):
                        nc.tensor.matmul(
                            ps_o,
                            lhsT=sb_ctxT[0:64, h, st * 128:(st + 1) * 128],
                            rhs=sb_wo[0:64, h, nb2 * 512:(nb2 + 1) * 512],
                            start=(h == 0), stop=(h == HPC - 1))
                    nc.scalar.copy(out=t_out[:, nb2 * 512:(nb2 + 1) * 512],
                                   in_=ps_o)
                nc.gpsimd.dma_start(out[st * 128:(st + 1) * 128, :], t_out)


def _prep_core_inputs(query, key, value, Wq, Wk, Wv, Wo, bq, bk, bv, aug):
    """Build the 8 per-core input maps (host-side shard + transpose + cast)."""
    in_maps = []
    if aug:
        aug_blk = np.zeros((128, S), np.float32)
        aug_blk[0, :] = 1.0
    for b in range(B):
        qTb = query[b].T
        kTb = key[b].T
        vTb = value[b].T
        if aug:
            qTb = np.concatenate([qTb, aug_blk], axis=0)
            kTb = np.concatenate([kTb, aug_blk], axis=0)
            vTb = np.concatenate([vTb, aug_blk], axis=0)
        qTb = np.ascontiguousarray(qTb).astype(BF16)
        kTb = np.ascontiguousarray(kTb).astype(BF16)
        vTb = np.ascontiguousarray(vTb).astype(BF16)
        for g in range(2):
            rows = slice(g * DHG, (g + 1) * DHG)
            wq_t = Wq[rows, :].T
            wk_t = Wk[rows, :].T
            wv_t = Wv[rows, :].T
            if aug:
                wq_t = np.concatenate(
                    [wq_t, np.concatenate([bq[None, rows],
                                           np.zeros((127, DHG), np.float32)])])
                wk_t = np.concatenate(
                    [wk_t, np.concatenate([bk[None, rows],
                                           np.zeros((127, DHG), np.float32)])])
                wv_t = np.concatenate(
                    [wv_t, np.concatenate([bv[None, rows],
                                           np.zeros((127, DHG), np.float32)])])
            # woTh[p, h, n] = Wo[n, g*512 + h*64 + p]
            wo_g = Wo[:, g * DHG:(g + 1) * DHG]          # [1024, 512]
            woTh = np.ascontiguousarray(
                wo_g.T.reshape(HPC, HEAD_DIM, D_MODEL).transpose(1, 0, 2))
            in_maps.append({
                "qT": qTb,
                "kT": kTb,
                "vT": vTb,
                "wqT": np.ascontiguousarray(wq_t).astype(BF16),
                "wkT": np.ascontiguousarray(wk_t).astype(BF16),
                "wvT": np.ascontiguousarray(wv_t).astype(BF16),
                "woTh": woTh.astype(BF16),
            })
    return in_maps


def kernel(**inputs):
    global LAST_RESULT
    query = np.asarray(inputs["query"], np.float32)
    key = np.asarray(inputs["key"], np.float32)
    value = np.asarray(inputs["value"], np.float32)
    Wq = np.asarray(inputs["Wq"], np.float32)
    Wk = np.asarray(inputs["Wk"], np.float32)
    Wv = np.asarray(inputs["Wv"], np.float32)
    Wo = np.asarray(inputs["Wo"], np.float32)
    bq = np.asarray(inputs["bq"], np.float32)
    bk = np.asarray(inputs["bk"], np.float32)
    bv = np.asarray(inputs["bv"], np.float32)
    bo = np.asarray(inputs["bo"], np.float32)

    aug = bool(np.any(bq) or np.any(bk) or np.any(bv))
    kt = 9 if aug else 8
    nc = _PROGRAM_CACHE.get(kt)
    if nc is None:
        nc = _build_program(kt)
        _PROGRAM_CACHE[kt] = nc

    in_maps = _prep_core_inputs(query, key, value, Wq, Wk, Wv, Wo,
                                bq, bk, bv, aug)
    res = run_bass_kernel_spmd(
        nc, in_maps, core_ids=list(range(8)),
        trace=TRACE,
        **({"trace_cores": TRACE_CORES} if TRACE_CORES else {}))
    LAST_RESULT = res

    out = np.empty((B, S, D_MODEL), np.float32)
    for b in range(B):
        out[b] = res.results[2 * b]["out"] + res.results[2 * b + 1]["out"] + bo
    return out


# revision 13
# speedup vs baseline: 1.1637x; 1.1637x over previous
"""MultiHeadAttention Trainium2 kernel (B=4, S=2048, D=1024, H=16, Dh=64).

Sharding: 8 cores = 4 batches x 2 head-groups (8 heads each).  Each core
computes QKV projections for its (batch, head-group), full attention for its
8 heads, and a partial output projection (row-parallel over Wo).  The host
sums the two per-batch partials and adds the output bias.

Per-core dataflow (all matmuls bf16 inputs, fp32 PSUM accumulation):
  - inputs arrive pre-transposed: qT/kT/vT [D, S], weight slices transposed.
  - QT/KT [512, S] computed directly in transposed (head-dim major) layout.
  - V [S, 512] computed in natural layout, stored with a ones-column per head
    (V_ext [s, h, 65]) so the attention-value matmul also produces the
    softmax denominator.
  - scoresT[k, q] = K_h @ Q_h^T   (contraction over head dim, K=64)
  - expT = exp(scoresT / 8) on ScalarE (no max subtraction: |scores/8| < ~7)
  - ctxT_ext[65, q] = V_ext^T @ expT accumulated over k tiles; row 64 is the
    denominator.  reciprocal -> PE broadcast over 64 partitions -> multiply.
  - out_partial[s, 1024] accumulated per head (K=64) from ctxT and WoT.
"""

import sys

for _p in ("/opt/trn_rl_repo", "/root/.axon_site/_ro/trn_rl_repo"):
    if _p not in sys.path:
        sys.path.append(_p)

import numpy as np
import ml_dtypes

import concourse.bass as bass
import concourse.tile as tile
from concourse import bacc, mybir
from concourse.bass_utils import run_bass_kernel_spmd

BF16 = ml_dtypes.bfloat16
F32 = mybir.dt.float32
BF = mybir.dt.bfloat16

D_MODEL = 1024
NUM_HEADS = 16
HEAD_DIM = 64
B, S = 4, 2048
HPC = 8          # heads per core
DHG = HPC * HEAD_DIM  # 512, head dims per core

# knobs read by test.py
TRACE = False
TRACE_CORES = None
LAST_RESULT = None

_PROGRAM_CACHE = {}


def _build_program(kt: int) -> bass.Bass:
    """Build the per-core SPMD program. kt = contraction tiles over d_model
    (8 normally, 9 when inputs are augmented with a bias row)."""
    nc = bacc.Bacc("TRN2", debug=False, target_bir_lowering=False)

    kd = kt * 128
    qT = nc.dram_tensor("qT", [kd, S], BF, kind="ExternalInput").ap()
    kT = nc.dram_tensor("kT", [kd, S], BF, kind="ExternalInput").ap()
    vT = nc.dram_tensor("vT", [kd, S], BF, kind="ExternalInput").ap()
    wqT = nc.dram_tensor("wqT", [kd, DHG], BF, kind="ExternalInput").ap()
    wkT = nc.dram_tensor("wkT", [kd, DHG], BF, kind="ExternalInput").ap()
    wvT = nc.dram_tensor("wvT", [kd, DHG], BF, kind="ExternalInput").ap()
    woTh = nc.dram_tensor("woTh", [HEAD_DIM, HPC, D_MODEL], BF,
                          kind="ExternalInput").ap()
    out = nc.dram_tensor("out", [S, D_MODEL], F32, kind="ExternalOutput").ap()

    with tile.TileContext(nc) as tc:
        _body(tc, qT, kT, vT, wqT, wkT, wvT, woTh, out, kt)
    nc.compile()
    return nc


def _body(tc, qT, kT, vT, wqT, wkT, wvT, woTh, out, kt):
    nc = tc.nc
    EXP = mybir.ActivationFunctionType.Exp

    with (
        tc.tile_pool(name="consts", bufs=1) as consts,
        tc.tile_pool(name="bigs", bufs=1) as bigs,
    ):
        # ---- resident tensors ------------------------------------------------
        sb_wq = consts.tile([128, kt, DHG], BF)
        sb_wk = consts.tile([128, kt, DHG], BF)
        sb_wv = consts.tile([128, kt, DHG], BF)
        sb_wo = consts.tile([HEAD_DIM, HPC, D_MODEL], BF)
        nc.sync.dma_start(sb_wq, wqT.rearrange("(t p) m -> p t m", p=128))
        nc.sync.dma_start(sb_wk, wkT.rearrange("(t p) m -> p t m", p=128))
        nc.sync.dma_start(sb_wv, wvT.rearrange("(t p) m -> p t m", p=128))
        nc.sync.dma_start(sb_wo, woTh)

        sb_QT = bigs.tile([128, 4, S], BF)    # [dh%128, dh//128, s]
        sb_KT = bigs.tile([128, 4, S], BF)
        sb_V = bigs.tile([128, 16, HPC, 65], BF)  # [s%128, s//128, h, dh+den]
        sb_ctxT = bigs.tile([64, HPC, S], BF)     # [dh%64, h, s]
        nc.vector.memset(sb_V[:, :, :, 64:65], 1.0)

        # ---- stage B: projections -------------------------------------------
        # Q and K: out[dh-tile 128, s] ; contraction over d_model.
        for name, w_t, src, dst, eng in (
            ("q", sb_wq, qT, sb_QT, nc.scalar),
            ("k", sb_wk, kT, sb_KT, nc.vector),
        ):
            with (
                tc.tile_pool(name=f"ld_{name}", bufs=2 * kt) as loads,
                tc.tile_pool(name=f"ps_{name}", bufs=8, space="PSUM") as psums,
            ):
                for nbp in range(2):            # s blocks of 1024
                    ps = [psums.tile([128, 512], F32, tag="pp", name=f"pp{name}{nbp}_{i}")
                          for i in range(8)]
                    for ki in range(kt):
                        t_in = loads.tile([128, 1024], BF, tag="ld")
                        nc.sync.dma_start(
                            t_in,
                            src[ki * 128:(ki + 1) * 128,
                                nbp * 1024:(nbp + 1) * 1024])
                        for nb in range(2):
                            for g in range(4):
                                nc.tensor.matmul(
                                    ps[nb * 4 + g],
                                    lhsT=w_t[:, ki, g * 128:(g + 1) * 128],
                                    rhs=t_in[:, nb * 512:(nb + 1) * 512],
                                    start=(ki == 0), stop=(ki == kt - 1))
                    for nb in range(2):
                        for g in range(4):
                            c0 = nbp * 1024 + nb * 512
                            if eng is nc.scalar:
                                eng.copy(out=dst[:, g, c0:c0 + 512],
                                         in_=ps[nb * 4 + g])
                            else:
                                eng.tensor_copy(out=dst[:, g, c0:c0 + 512],
                                                in_=ps[nb * 4 + g])

        # V: out[s-tile 128, dh 512] ; lhsT = input tile (stationary).
        with (
            tc.tile_pool(name="ld_v", bufs=2 * kt) as loads,
            tc.tile_pool(name="ps_v", bufs=8, space="PSUM") as psums,
        ):
            for nbp in range(2):
                ps = [psums.tile([128, 512], F32, tag="pv", name=f"pv{nbp}_{i}") for i in range(8)]
                for ki in range(kt):
                    t_in = loads.tile([128, 1024], BF, tag="lv")
                    nc.sync.dma_start(
                        t_in,
                        vT[ki * 128:(ki + 1) * 128,
                           nbp * 1024:(nbp + 1) * 1024])
                    for sti in range(8):
                        nc.tensor.matmul(
                            ps[sti],
                            lhsT=t_in[:, sti * 128:(sti + 1) * 128],
                            rhs=sb_wv[:, ki, :],
                            start=(ki == 0), stop=(ki == kt - 1))
                for sti in range(8):
                    st = nbp * 8 + sti
                    nc.vector.tensor_copy(
                        out=sb_V[:, st, :, 0:64],
                        in_=ps[sti].rearrange("p (h d) -> p h d", h=HPC))

        # ---- stage C: attention ---------------------------------------------
        with (
            tc.tile_pool(name="exps", bufs=6) as exps,
            tc.tile_pool(name="smalls", bufs=3) as smalls,
            tc.tile_pool(name="ps_sc", bufs=3, space="PSUM") as ps_sc_pool,
            tc.tile_pool(name="ps_cx", bufs=2, space="PSUM") as ps_cx_pool,
        ):
            for h in range(HPC):
                pi0 = (h % 2) * 64
                g = h // 2
                for qb in range(4):          # query blocks of 512
                    q0 = qb * 512
                    ps_ctx = ps_cx_pool.tile([128, 512], F32, tag="ctx")
                    for kp in range(8):      # pairs of key tiles
                        ps_sc = ps_sc_pool.tile([128, 1024], F32, tag="sc")
                        for j in range(2):
                            ktile = kp * 2 + j
                            nc.tensor.matmul(
                                ps_sc[:, j * 512:(j + 1) * 512],
                                lhsT=sb_KT[pi0:pi0 + 64, g,
                                           ktile * 128:(ktile + 1) * 128],
                                rhs=sb_QT[pi0:pi0 + 64, g, q0:q0 + 512],
                                start=True, stop=True)
                        t_e = exps.tile([128, 1024], BF, tag="exp")
                        nc.scalar.activation(t_e, ps_sc, EXP, scale=0.125)
                        for j in range(2):
                            ktile = kp * 2 + j
                            nc.tensor.matmul(
                                ps_ctx[0:65, :],
                                lhsT=sb_V[:, ktile, h, :],
                                rhs=t_e[:, j * 512:(j + 1) * 512],
                                start=(ktile == 0), stop=(ktile == 15))
                    # normalize: row 64 of ps_ctx holds the softmax denominator.
                    # Chain is entirely off the PE stream; partition_broadcast
                    # only works from physical partition 0, so DMA-move the
                    # row there first.
                    t_rd = smalls.tile([128, 512], F32, tag="rd")
                    nc.vector.tensor_copy(out=t_rd[64:65, :],
                                          in_=ps_ctx[64:65, :])
                    nc.sync.dma_start(t_rd[0:1, :], t_rd[64:65, :])
                    t_rc = smalls.tile([1, 512], F32, tag="rc")
                    nc.vector.reciprocal(t_rc, t_rd[0:1, :])
                    t_rdb = smalls.tile([64, 512], F32, tag="rdb")
                    nc.gpsimd.partition_broadcast(t_rdb, t_rc)
                    nc.vector.tensor_mul(
                        out=sb_ctxT[0:64, h, q0:q0 + 512],
                        in0=ps_ctx[0:64, :],
                        in1=t_rdb)

        # ---- stage D: output projection (partial; host adds the halves) -----
        with (
            tc.tile_pool(name="outst", bufs=16) as outs_pool,
            tc.tile_pool(name="ps_o", bufs=4, space="PSUM") as ps_o_pool,
        ):
            for st in range(16):
                t_out = outs_pool.tile([128, D_MODEL], F32, tag="o")
                for nb2 in range(2):
                    ps_o = ps_o_pool.tile([128, 512], F32, tag="po")
                    for h in range(HPC):
                        nc.tensor.matmul(
                            ps_o,
                            lhsT=sb_ctxT[0:64, h, st * 128:(st + 1) * 128],
                            rhs=sb_wo[0:64, h, nb2 * 512:(nb2 + 1) * 512],
                            start=(h == 0), stop=(h == HPC - 1))
                    nc.scalar.copy(out=t_out[:, nb2 * 512:(nb2 + 1) * 512],
                                   in_=ps_o)
                nc.sync.dma_start(out[st * 128:(st + 1) * 128, :], t_out)


def _prep_core_inputs(query, key, value, Wq, Wk, Wv, Wo, bq, bk, bv, aug):
    """Build the 8 per-core input maps (host-side shard + transpose + cast)."""
    in_maps = []
    if aug:
        aug_blk = np.zeros((128, S), np.float32)
        aug_blk[0, :] = 1.0
    for b in range(B):
        qTb = query[b].T
        kTb = key[b].T
        vTb = value[b].T
        if aug:
            qTb = np.concatenate([qTb, aug_blk], axis=0)
            kTb = np.concatenate([kTb, aug_blk], axis=0)
            vTb = np.concatenate([vTb, aug_blk], axis=0)
        qTb = np.ascontiguousarray(qTb).astype(BF16)
        kTb = np.ascontiguousarray(kTb).astype(BF16)
        vTb = np.ascontiguousarray(vTb).astype(BF16)
        for g in range(2):
            rows = slice(g * DHG, (g + 1) * DHG)
            wq_t = Wq[rows, :].T
            wk_t = Wk[rows, :].T
            wv_t = Wv[rows, :].T
            if aug:
                wq_t = np.concatenate(
                    [wq_t, np.concatenate([bq[None, rows],
                                           np.zeros((127, DHG), np.float32)])])
                wk_t = np.concatenate(
                    [wk_t, np.concatenate([bk[None, rows],
                                           np.zeros((127, DHG), np.float32)])])
                wv_t = np.concatenate(
                    [wv_t, np.concatenate([bv[None, rows],
                                           np.zeros((127, DHG), np.float32)])])
            # woTh[p, h, n] = Wo[n, g*512 + h*64 + p]
            wo_g = Wo[:, g * DHG:(g + 1) * DHG]          # [1024, 512]
            woTh = np.ascontiguousarray(
                wo_g.T.reshape(HPC, HEAD_DIM, D_MODEL).transpose(1, 0, 2))
            in_maps.append({
                "qT": qTb,
                "kT": kTb,
                "vT": vTb,
                "wqT": np.ascontiguousarray(wq_t).astype(BF16),
                "wkT": np.ascontiguousarray(wk_t).astype(BF16),
                "wvT": np.ascontiguousarray(wv_t).astype(BF16),
                "woTh": woTh.astype(BF16),
            })
    return in_maps


def kernel(**inputs):
    global LAST_RESULT
    query = np.asarray(inputs["query"], np.float32)
    key = np.asarray(inputs["key"], np.float32)
    value = np.asarray(inputs["value"], np.float32)
    Wq = np.asarray(inputs["Wq"], np.float32)
    Wk = np.asarray(inputs["Wk"], np.float32)
    Wv = np.asarray(inputs["Wv"], np.float32)
    Wo = np.asarray(inputs["Wo"], np.float32)
    bq = np.asarray(inputs["bq"], np.float32)
    bk = np.asarray(inputs["bk"], np.float32)
    bv = np.asarray(inputs["bv"], np.float32)
    bo = np.asarray(inputs["bo"], np.float32)

    aug = bool(np.any(bq) or np.any(bk) or np.any(bv))
    kt = 9 if aug else 8
    nc = _PROGRAM_CACHE.get(kt)
    if nc is None:
        nc = _build_program(kt)
        _PROGRAM_CACHE[kt] = nc

    in_maps = _prep_core_inputs(query, key, value, Wq, Wk, Wv, Wo,
                                bq, bk, bv, aug)
    res = run_bass_kernel_spmd(
        nc, in_maps, core_ids=list(range(8)),
        trace=TRACE,
        **({"trace_cores": TRACE_CORES} if TRACE_CORES else {}))
    LAST_RESULT = res

    out = np.empty((B, S, D_MODEL), np.float32)
    for b in range(B):
        out[b] = res.results[2 * b]["out"] + res.results[2 * b + 1]["out"] + bo
    return out


# revision 14
# speedup vs baseline: 1.7762x; 1.5263x over previous
"""MultiHeadAttention Trainium2 kernel (B=4, S=2048, D=1024, H=16, Dh=64).

Sharding: 8 cores = 4 batches x 2 head-groups (8 heads each).  Each core
computes QKV projections for its (batch, head-group), full attention for its
8 heads, and a partial output projection (row-parallel over Wo).  The host
sums the two per-batch partials and adds the output bias.

Per-core dataflow (all matmuls bf16 inputs, fp32 PSUM accumulation):
  - inputs arrive pre-transposed: qT/kT/vT [D, S], weight slices transposed.
  - QT/KT [512, S] computed directly in transposed (head-dim major) layout.
  - V [S, 512] computed in natural layout, stored with a ones-column per head
    (V_ext [s, h, 65]) so the attention-value matmul also produces the
    softmax denominator.
  - scoresT[k, q] = K_h @ Q_h^T   (contraction over head dim, K=64)
  - expT = exp(scoresT / 8) on ScalarE (no max subtraction: |scores/8| < ~7)
  - ctxT_ext[65, q] = V_ext^T @ expT accumulated over k tiles; row 64 is the
    denominator.  reciprocal -> PE broadcast over 64 partitions -> multiply.
  - out_partial[s, 1024] accumulated per head (K=64) from ctxT and WoT.
"""

import sys

for _p in ("/opt/trn_rl_repo", "/root/.axon_site/_ro/trn_rl_repo"):
    if _p not in sys.path:
        sys.path.append(_p)

import numpy as np
import ml_dtypes

import concourse.bass as bass
import concourse.tile as tile
from concourse import bacc, mybir
from concourse.bass_utils import run_bass_kernel_spmd

BF16 = ml_dtypes.bfloat16
F32 = mybir.dt.float32
BF = mybir.dt.bfloat16

D_MODEL = 1024
NUM_HEADS = 16
HEAD_DIM = 64
B, S = 4, 2048
HPC = 8          # heads per core
DHG = HPC * HEAD_DIM  # 512, head dims per core

# knobs read by test.py
TRACE = False
TRACE_CORES = None
LAST_RESULT = None

_PROGRAM_CACHE = {}


def _build_program(kt: int) -> bass.Bass:
    """Build the per-core SPMD program. kt = contraction tiles over d_model
    (8 normally, 9 when inputs are augmented with a bias row)."""
    nc = bacc.Bacc("TRN2", debug=False, target_bir_lowering=False)

    kd = kt * 128
    qT = nc.dram_tensor("qT", [kd, S], BF, kind="ExternalInput").ap()
    kT = nc.dram_tensor("kT", [kd, S], BF, kind="ExternalInput").ap()
    vT = nc.dram_tensor("vT", [kd, S], BF, kind="ExternalInput").ap()
    wqT = nc.dram_tensor("wqT", [kd, DHG], BF, kind="ExternalInput").ap()
    wkT = nc.dram_tensor("wkT", [kd, DHG], BF, kind="ExternalInput").ap()
    wvT = nc.dram_tensor("wvT", [kd, DHG], BF, kind="ExternalInput").ap()
    woTh = nc.dram_tensor("woTh", [HEAD_DIM, HPC, D_MODEL], BF,
                          kind="ExternalInput").ap()
    out = nc.dram_tensor("out", [S, D_MODEL], F32, kind="ExternalOutput").ap()

    with tile.TileContext(nc) as tc:
        _body(tc, qT, kT, vT, wqT, wkT, wvT, woTh, out, kt)
    nc.compile()
    return nc


def _body(tc, qT, kT, vT, wqT, wkT, wvT, woTh, out, kt):
    nc = tc.nc
    EXP = mybir.ActivationFunctionType.Exp

    with (
        tc.tile_pool(name="consts", bufs=1) as consts,
        tc.tile_pool(name="bigctx", bufs=1) as bigctx,
    ):
        # ---- resident tensors ------------------------------------------------
        # Attention/out-proj matmuls are padded to a full K=128 contraction
        # with explicit zero rows: half-array (K=64) matmuls never satisfy the
        # PE activity monitor, pinning the clock at 1.2 GHz.  Padding is free
        # (matmul time is set by the moving free dim, not K).
        sb_wq = consts.tile([128, kt, DHG], BF)
        sb_wk = consts.tile([128, kt, DHG], BF)
        sb_wv = consts.tile([128, kt, DHG], BF)
        sb_wo = consts.tile([128, HPC, D_MODEL], BF)
        nc.sync.dma_start(sb_wq, wqT.rearrange("(t p) m -> p t m", p=128))
        nc.sync.dma_start(sb_wk, wkT.rearrange("(t p) m -> p t m", p=128))
        nc.sync.dma_start(sb_wv, wvT.rearrange("(t p) m -> p t m", p=128))
        nc.vector.memset(sb_wo[64:128, :, :], 0.0)
        nc.sync.dma_start(sb_wo[0:64, :, :], woTh)

        # ctxT: rows 0:64 hold head h's context, rows 64:128 stay zero.
        sb_ctxT = bigctx.tile([128, HPC, S], BF)
        nc.vector.memset(sb_ctxT, 0.0)

        with tc.tile_pool(name="bigqkv", bufs=1) as bigqkv:
            # per-head slots; head h's data occupies partitions (h%2)*64 ..
            # +64 of slot h, the other 64 partitions are zero.
            sb_QT = bigqkv.tile([128, HPC, S], BF)
            sb_KT = bigqkv.tile([128, HPC, S], BF)
            sb_V = bigqkv.tile([128, 16, HPC, 65], BF)  # [s%128, s//128, h, :]
            nc.vector.memset(sb_QT, 0.0)
            nc.vector.memset(sb_KT, 0.0)
            nc.vector.memset(sb_V[:, :, :, 64:65], 1.0)

            # ---- stage B: projections ---------------------------------------
            # Q and K: psum [dh-tile 128, s] holds heads (2g, 2g+1).
            for name, w_t, src, dst, eng in (
                ("q", sb_wq, qT, sb_QT, nc.scalar),
                ("k", sb_wk, kT, sb_KT, nc.vector),
            ):
                with (
                    tc.tile_pool(name=f"ld_{name}", bufs=2 * kt) as loads,
                    tc.tile_pool(name=f"ps_{name}", bufs=8,
                                 space="PSUM") as psums,
                ):
                    for nbp in range(2):        # s blocks of 1024
                        ps = [psums.tile([128, 512], F32, tag="pp",
                                         name=f"pp{name}{nbp}_{i}")
                              for i in range(8)]
                        for ki in range(kt):
                            t_in = loads.tile([128, 1024], BF, tag="ld")
                            nc.sync.dma_start(
                                t_in,
                                src[ki * 128:(ki + 1) * 128,
                                    nbp * 1024:(nbp + 1) * 1024])
                            for nb in range(2):
                                for g in range(4):
                                    nc.tensor.matmul(
                                        ps[nb * 4 + g],
                                        lhsT=w_t[:, ki, g * 128:(g + 1) * 128],
                                        rhs=t_in[:, nb * 512:(nb + 1) * 512],
                                        start=(ki == 0), stop=(ki == kt - 1))
                        for nb in range(2):
                            for g in range(4):
                                c0 = nbp * 1024 + nb * 512
                                p = ps[nb * 4 + g]
                                for par in range(2):    # head 2g / 2g+1
                                    h = 2 * g + par
                                    sl = slice(par * 64, par * 64 + 64)
                                    if eng is nc.scalar:
                                        eng.copy(
                                            out=dst[sl, h, c0:c0 + 512],
                                            in_=p[sl, :])
                                    else:
                                        eng.tensor_copy(
                                            out=dst[sl, h, c0:c0 + 512],
                                            in_=p[sl, :])

            # V: out[s-tile 128, dh 512] ; lhsT = input tile (stationary).
            with (
                tc.tile_pool(name="ld_v", bufs=2 * kt) as loads,
                tc.tile_pool(name="ps_v", bufs=8, space="PSUM") as psums,
            ):
                for nbp in range(2):
                    ps = [psums.tile([128, 512], F32, tag="pv",
                                     name=f"pv{nbp}_{i}") for i in range(8)]
                    for ki in range(kt):
                        t_in = loads.tile([128, 1024], BF, tag="lv")
                        nc.sync.dma_start(
                            t_in,
                            vT[ki * 128:(ki + 1) * 128,
                               nbp * 1024:(nbp + 1) * 1024])
                        for sti in range(8):
                            nc.tensor.matmul(
                                ps[sti],
                                lhsT=t_in[:, sti * 128:(sti + 1) * 128],
                                rhs=sb_wv[:, ki, :],
                                start=(ki == 0), stop=(ki == kt - 1))
                    for sti in range(8):
                        st = nbp * 8 + sti
                        nc.vector.tensor_copy(
                            out=sb_V[:, st, :, 0:64],
                            in_=ps[sti].rearrange("p (h d) -> p h d", h=HPC))

            # ---- stage C: attention -----------------------------------------
            with (
                tc.tile_pool(name="exps", bufs=6) as exps,
                tc.tile_pool(name="smalls", bufs=3) as smalls,
                tc.tile_pool(name="ps_sc", bufs=3, space="PSUM") as ps_sc_pool,
                tc.tile_pool(name="ps_cx", bufs=2, space="PSUM") as ps_cx_pool,
            ):
                for h in range(HPC):
                    for qb in range(4):      # query blocks of 512
                        q0 = qb * 512
                        ps_ctx = ps_cx_pool.tile([128, 512], F32, tag="ctx")
                        for kp in range(8):  # pairs of key tiles
                            ps_sc = ps_sc_pool.tile([128, 1024], F32, tag="sc")
                            for j in range(2):
                                ktile = kp * 2 + j
                                nc.tensor.matmul(
                                    ps_sc[:, j * 512:(j + 1) * 512],
                                    lhsT=sb_KT[:, h,
                                               ktile * 128:(ktile + 1) * 128],
                                    rhs=sb_QT[:, h, q0:q0 + 512],
                                    start=True, stop=True)
                            t_e = exps.tile([128, 1024], BF, tag="exp")
                            nc.scalar.activation(t_e, ps_sc, EXP, scale=0.125)
                            for j in range(2):
                                ktile = kp * 2 + j
                                nc.tensor.matmul(
                                    ps_ctx[0:65, :],
                                    lhsT=sb_V[:, ktile, h, :],
                                    rhs=t_e[:, j * 512:(j + 1) * 512],
                                    start=(ktile == 0), stop=(ktile == 15))
                        # normalize: row 64 of ps_ctx is the denominator.
                        # Chain stays off the PE stream; partition_broadcast
                        # only works from physical partition 0, so DMA-move
                        # the row there first.
                        t_rd = smalls.tile([128, 512], F32, tag="rd")
                        nc.vector.tensor_copy(out=t_rd[64:65, :],
                                              in_=ps_ctx[64:65, :])
                        nc.sync.dma_start(t_rd[0:1, :], t_rd[64:65, :])
                        t_rc = smalls.tile([1, 512], F32, tag="rc")
                        nc.vector.reciprocal(t_rc, t_rd[0:1, :])
                        t_rdb = smalls.tile([64, 512], F32, tag="rdb")
                        nc.gpsimd.partition_broadcast(t_rdb, t_rc)
                        nc.vector.tensor_mul(
                            out=sb_ctxT[0:64, h, q0:q0 + 512],
                            in0=ps_ctx[0:64, :],
                            in1=t_rdb)

        # ---- stage D: output projection (partial; host adds the halves) -----
        with (
            tc.tile_pool(name="outst", bufs=16) as outs_pool,
            tc.tile_pool(name="ps_o", bufs=4, space="PSUM") as ps_o_pool,
        ):
            for st in range(16):
                t_out = outs_pool.tile([128, D_MODEL], F32, tag="o")
                for nb2 in range(2):
                    ps_o = ps_o_pool.tile([128, 512], F32, tag="po")
                    for h in range(HPC):
                        nc.tensor.matmul(
                            ps_o,
                            lhsT=sb_ctxT[:, h, st * 128:(st + 1) * 128],
                            rhs=sb_wo[:, h, nb2 * 512:(nb2 + 1) * 512],
                            start=(h == 0), stop=(h == HPC - 1))
                    nc.scalar.copy(out=t_out[:, nb2 * 512:(nb2 + 1) * 512],
                                   in_=ps_o)
                nc.sync.dma_start(out[st * 128:(st + 1) * 128, :], t_out)


def _prep_core_inputs(query, key, value, Wq, Wk, Wv, Wo, bq, bk, bv, aug):
    """Build the 8 per-core input maps (host-side shard + transpose + cast)."""
    in_maps = []
    if aug:
        aug_blk = np.zeros((128, S), np.float32)
        aug_blk[0, :] = 1.0
    for b in range(B):
        qTb = query[b].T
        kTb = key[b].T
        vTb = value[b].T
        if aug:
            qTb = np.concatenate([qTb, aug_blk], axis=0)
            kTb = np.concatenate([kTb, aug_blk], axis=0)
            vTb = np.concatenate([vTb, aug_blk], axis=0)
        qTb = np.ascontiguousarray(qTb).astype(BF16)
        kTb = np.ascontiguousarray(kTb).astype(BF16)
        vTb = np.ascontiguousarray(vTb).astype(BF16)
        for g in range(2):
            rows = slice(g * DHG, (g + 1) * DHG)
            wq_t = Wq[rows, :].T
            wk_t = Wk[rows, :].T
            wv_t = Wv[rows, :].T
            if aug:
                wq_t = np.concatenate(
                    [wq_t, np.concatenate([bq[None, rows],
                                           np.zeros((127, DHG), np.float32)])])
                wk_t = np.concatenate(
                    [wk_t, np.concatenate([bk[None, rows],
                                           np.zeros((127, DHG), np.float32)])])
                wv_t = np.concatenate(
                    [wv_t, np.concatenate([bv[None, rows],
                                           np.zeros((127, DHG), np.float32)])])
            # woTh[p, h, n] = Wo[n, g*512 + h*64 + p]
            wo_g = Wo[:, g * DHG:(g + 1) * DHG]          # [1024, 512]
            woTh = np.ascontiguousarray(
                wo_g.T.reshape(HPC, HEAD_DIM, D_MODEL).transpose(1, 0, 2))
            in_maps.append({
                "qT": qTb,
                "kT": kTb,
                "vT": vTb,
                "wqT": np.ascontiguousarray(wq_t).astype(BF16),
                "wkT": np.ascontiguousarray(wk_t).astype(BF16),
                "wvT": np.ascontiguousarray(wv_t).astype(BF16),
                "woTh": woTh.astype(BF16),
            })
    return in_maps


def kernel(**inputs):
    global LAST_RESULT
    query = np.asarray(inputs["query"], np.float32)
    key = np.asarray(inputs["key"], np.float32)
    value = np.asarray(inputs["value"], np.float32)
    Wq = np.asarray(inputs["Wq"], np.float32)
    Wk = np.asarray(inputs["Wk"], np.float32)
    Wv = np.asarray(inputs["Wv"], np.float32)
    Wo = np.asarray(inputs["Wo"], np.float32)
    bq = np.asarray(inputs["bq"], np.float32)
    bk = np.asarray(inputs["bk"], np.float32)
    bv = np.asarray(inputs["bv"], np.float32)
    bo = np.asarray(inputs["bo"], np.float32)

    aug = bool(np.any(bq) or np.any(bk) or np.any(bv))
    kt = 9 if aug else 8
    nc = _PROGRAM_CACHE.get(kt)
    if nc is None:
        nc = _build_program(kt)
        _PROGRAM_CACHE[kt] = nc

    in_maps = _prep_core_inputs(query, key, value, Wq, Wk, Wv, Wo,
                                bq, bk, bv, aug)
    res = run_bass_kernel_spmd(
        nc, in_maps, core_ids=list(range(8)),
        trace=TRACE,
        **({"trace_cores": TRACE_CORES} if TRACE_CORES else {}))
    LAST_RESULT = res

    out = np.empty((B, S, D_MODEL), np.float32)
    for b in range(B):
        out[b] = res.results[2 * b]["out"] + res.results[2 * b + 1]["out"] + bo
    return out
